# revision 22
# baseline (speedup 1.0000x reference)
"""Llama attention layer (B=2, S=2048, H=4096, 32 heads, fp32 io) on 8 trn2 cores.

Sharding: tensor-parallel over heads. Each core owns 4 heads: W_qkv column
shard [4096, 3*512] (bf16), W_o row shard [512, 4096] (bf16). Each core
computes qkv proj + RoPE + causal attention for its heads + its o_proj
partial; the host sums the 8 fp32 partials (the "all-reduce").

Device kernel (per core), all matmuls bf16 with fp32 PSUM accumulation:
  phase 1: q/k/v = hiddenT-tiles.T @ W-chunks in natural [token, feat]
           layout; RoPE applied with free-dim slices; results bounced to
           DRAM scratch.
  phase 2: per (batch, head): load Q^T/K^T via DMA-transpose, V natural.
           S^T[k,q] = K^T-tile.T @ Q^T ; P = exp(SCALING*S) (no max-sub:
           |scores|<~6 so exp is safe in fp32) ; causal handled by
           multiplicative masks on 4 diagonal block offsets ; attn^T =
           V.T @ P^T accumulated over k-chunks; row sums l via ones-matmul;
           normalize by broadcast reciprocal.
  phase 3: o_partial[t, :] = attn^T-tiles.T @ W_o shard, accumulated over
           the 4 head-chunks, written as fp32.
"""

import numpy as np
import ml_dtypes

import concourse.bass as bass
import concourse.tile as tile
from concourse import bacc, mybir
from concourse.bass_utils import run_bass_kernel_spmd

# ---- problem constants (hardcoded per contract) ----
HIDDEN = 4096
NH = 32
D = 128
B = 2
S = 2048
TOK = B * S            # 4096 tokens
N_CORES = 8
HC = NH // N_CORES     # 4 heads per core
FH = HC * D            # 512 features per core for each of q/k/v
SCALING = D ** -0.5
ROPE_BASE = 10000.0

BF16 = mybir.dt.bfloat16
F32 = mybir.dt.float32

TBLK = 512             # tokens per phase-1 block
NTB = TOK // TBLK      # 8
HKC = 16               # h-dim 128-chunks per hT/w chunk tile (2 chunks = full 4096)
QT = 512               # q columns per phase-2 tile
NQT = S // QT          # 4
NKC = S // 128         # 16 k chunks per sequence


def _emit_phase1_block(nc, T, pools, dram):
    """One 512-token block: q/k/v projections (natural layout) + rope."""
    hp, wp, csp, rtp, stp, psp = (
        pools["hblk"], pools["wch"], pools["cs"], pools["rtmp"],
        pools["stage"], pools["ps"],
    )
    hT, w, csn = dram["hT"], dram["w"], dram["csn"]
    scr = [dram["q_nat"], dram["k_nat"], dram["v_nat"]]

    # hT block: [4096 h, 512 t] as 2 chunks of [128 p, 16 kc, 512 t]
    hblk = []
    for i in range(2):
        t_ = hp.tile([128, HKC, TBLK], BF16, tag="hblk")
        nc.sync.dma_start(
            out=t_,
            in_=hT[i * 2048:(i + 1) * 2048, T * TBLK:(T + 1) * TBLK].rearrange(
                "(kc p) t -> p kc t", p=128),
        )
        hblk.append(t_)

    # cos/sin tiles per token subtile: [128 t, 2, 4, 128]
    csts = []
    for tt in range(4):
        cst = csp.tile([128, 2, HC, D], BF16, tag="cs")
        r0 = T * TBLK + tt * 128
        nc.sync.dma_start(
            out=cst,
            in_=csn[r0:r0 + 128, :, :].rearrange("p c (h d) -> p c h d", h=HC),
        )
        csts.append(cst)

    for j3 in range(3):  # q, k, v
        # w chunk: [4096 h, 512 f] as 2 chunks of [128 p, 16 kc, 512 f]
        wch = []
        for i in range(2):
            t_ = wp.tile([128, HKC, FH], BF16, tag="wch")
            nc.sync.dma_start(
                out=t_,
                in_=w[i * 2048:(i + 1) * 2048, j3 * FH:(j3 + 1) * FH].rearrange(
                    "(kc p) f -> p kc f", p=128),
            )
            wch.append(t_)

        for tt in range(4):
            ps = psp.tile([128, HC, D], F32, tag="ps")
            for i in range(2):
                for kc in range(HKC):
                    nc.tensor.matmul(
                        ps,
                        lhsT=hblk[i][:, kc, tt * 128:(tt + 1) * 128],
                        rhs=wch[i][:, kc, :],
                        start=(i == 0 and kc == 0),
                        stop=(i == 1 and kc == HKC - 1),
                    )
            st = stp.tile([128, HC, D], BF16, tag="stage")
            if j3 < 2:
                cst = csts[tt]
                half = D // 2
                tr = rtp.tile([128, HC, D], F32, tag="rtmp")
                tcos = rtp.tile([128, HC, D], F32, tag="rtmp")
                # rotate-half times signed sin
                nc.vector.tensor_mul(tr[:, :, 0:half], ps[:, :, half:D], cst[:, 1, :, 0:half])
                nc.vector.tensor_mul(tr[:, :, half:D], ps[:, :, 0:half], cst[:, 1, :, half:D])
                nc.vector.tensor_mul(tcos, ps, cst[:, 0])
                nc.vector.tensor_add(st, tr, tcos)
            else:
                nc.vector.tensor_copy(out=st, in_=ps)
            r0 = T * TBLK + tt * 128
            nc.sync.dma_start(out=scr[j3][r0:r0 + 128, :], in_=st)


def _emit_phase2_pair(nc, b, hh, pools, dram, attn_s, mskt, ones_col,
                      qt_hook=None):
    """Causal attention for one (batch, local-head) pair.

    k-chunks processed in groups of 2 so one ACTIVATE covers [128, 2*QT]
    (amortizes the ~400ns per-instruction ACT overhead)."""
    pairp, ptp, pvlp, ps2p, smallp = (
        pools["pair"], pools["pt"], pools["pvl"], pools["ps2"], pools["small"],
    )
    q_nat, k_nat, v_nat = dram["q_nat"], dram["k_nat"], dram["v_nat"]
    rows = slice(b * S, (b + 1) * S)
    cols = slice(hh * D, (hh + 1) * D)

    qTp = pairp.tile([128, S], BF16, tag="pair")
    nc.sync.dma_start_transpose(out=qTp, in_=q_nat[rows, cols])
    kTp = pairp.tile([128, S], BF16, tag="pair")
    nc.sync.dma_start_transpose(out=kTp, in_=k_nat[rows, cols])
    vp = pairp.tile([128, NKC, D], BF16, tag="pair")
    nc.sync.dma_start(
        out=vp, in_=v_nat[rows, cols].rearrange("(kc p) d -> p kc d", p=128))

    for qt in range(NQT):
        qcols = slice(qt * QT, (qt + 1) * QT)
        pv = pvlp.tile([128, QT], F32, tag="pvl")
        pl = pvlp.tile([128, QT], F32, tag="pvl", name="pl")
        nkc = 4 * (qt + 1)
        for g in range(nkc // 2):
            sc = ps2p.tile([128, 2, QT], F32, tag="ps2")
            for s2 in range(2):
                kc = 2 * g + s2
                nc.tensor.matmul(
                    sc[:, s2, :], lhsT=kTp[:, kc * 128:(kc + 1) * 128],
                    rhs=qTp[:, qcols], start=True, stop=True)
            pt2 = ptp.tile([128, 2, QT], BF16, tag="pt")
            nc.scalar.activation(
                out=pt2, in_=sc, func=mybir.ActivationFunctionType.Exp,
                scale=SCALING)
            if g >= 2 * qt:  # diagonal group: apply causal mask pair
                pt2m = ptp.tile([128, 2, QT], BF16, tag="pt")
                nc.vector.tensor_mul(pt2m, pt2, mskt[:, g - 2 * qt])
                pt2 = pt2m
            for s2 in range(2):
                kc = 2 * g + s2
                nc.tensor.matmul(
                    pv, lhsT=vp[:, kc, :], rhs=pt2[:, s2, :],
                    start=(kc == 0), stop=(kc == nkc - 1))
                nc.tensor.matmul(
                    pl[0:1, :], lhsT=ones_col, rhs=pt2[:, s2, :],
                    start=(kc == 0), stop=(kc == nkc - 1))
        # normalize: attn^T[:, qcols] = pv * (1/l) broadcast over partitions
        linv = smallp.tile([1, QT], F32, tag="linv1")
        nc.vector.reciprocal_approx_fast(out=linv, in_=pl[0:1, :])
        linv_bc = smallp.tile([128, QT], F32, tag="linvbc")
        nc.gpsimd.partition_broadcast(linv_bc, linv)
        nc.vector.tensor_mul(
            attn_s[b][:, hh, qt * QT:(qt + 1) * QT], pv, linv_bc)
        if qt_hook is not None:
            qt_hook(qt)


def _emit_phase3_wo(nc, pools, dram):
    wot = pools["wo"].tile([128, HC, HIDDEN], BF16, tag="wo")
    nc.sync.dma_start(out=wot, in_=dram["wo"].rearrange("(kc p) o -> p kc o", p=128))
    return wot


def _emit_phase3_sts(nc, pools, dram, attn_s, wot, sts):
    """o_proj partial for a range of 128-token tiles."""
    ostp, psp = pools["ost"], pools["ps"]
    outp = dram["outp"]

    for st in sts:
        for ocp in range(4):  # pairs of 512-col output tiles
            pso = [psp.tile([128, QT], F32, tag="ps", name="pso0"),
                   psp.tile([128, QT], F32, tag="ps", name="pso1")]
            for kc in range(HC):
                for oc in range(2):
                    o0 = (ocp * 2 + oc) * QT
                    nc.tensor.matmul(
                        pso[oc],
                        lhsT=attn_s[st // 16][:, kc, (st % 16) * 128:
                                              (st % 16 + 1) * 128],
                        rhs=wot[:, kc, o0:o0 + QT],
                        start=(kc == 0), stop=(kc == HC - 1))
            for oc in range(2):
                o0 = (ocp * 2 + oc) * QT
                ot = ostp.tile([128, QT], F32, tag="ost")
                if oc == 0:
                    nc.vector.tensor_copy(out=ot, in_=pso[oc])
                else:
                    nc.scalar.copy(out=ot, in_=pso[oc])
                nc.sync.dma_start(
                    out=outp[st * 128:(st + 1) * 128, o0:o0 + QT], in_=ot)


def build_nc():
    nc = bacc.Bacc("TRN2", target_bir_lowering=False, debug=False,
                   num_devices=N_CORES)
    dram = {
        "hT": nc.dram_tensor("hT", [HIDDEN, TOK], BF16, kind="ExternalInput").ap(),
        "w": nc.dram_tensor("w", [HIDDEN, 3 * FH], BF16, kind="ExternalInput").ap(),
        "wo": nc.dram_tensor("wo", [FH, HIDDEN], BF16, kind="ExternalInput").ap(),
        "csn": nc.dram_tensor("csn", [TOK, 2, FH], BF16, kind="ExternalInput").ap(),
        "msk": nc.dram_tensor("msk", [128, 2, 2, QT], BF16, kind="ExternalInput").ap(),
        "outp": nc.dram_tensor("outp", [TOK, HIDDEN], F32, kind="ExternalOutput").ap(),
    }

    with tile.TileContext(nc) as tc:
        from contextlib import ExitStack
        with ExitStack() as ctx:
            # global pools (live across phases)
            pools = {}
            pools["ps"] = ctx.enter_context(tc.tile_pool(name="ps", bufs=4, space="PSUM"))
            pools["pvl"] = ctx.enter_context(tc.tile_pool(name="pvl", bufs=2, space="PSUM"))
            pools["ps2"] = ctx.enter_context(tc.tile_pool(name="ps2", bufs=1, space="PSUM"))
            gsb = ctx.enter_context(tc.tile_pool(name="gsb", bufs=1))
            pools["pair"] = ctx.enter_context(tc.tile_pool(name="pair", bufs=6))
            pools["pt"] = ctx.enter_context(tc.tile_pool(name="pt", bufs=4))
            pools["small"] = ctx.enter_context(tc.tile_pool(name="small", bufs=2))
            dscr = ctx.enter_context(tc.tile_pool(name="dscr", bufs=1, space="DRAM"))

            dram["q_nat"] = dscr.tile([TOK, FH], BF16, tag="qs", name="q_nat")
            dram["k_nat"] = dscr.tile([TOK, FH], BF16, tag="ks", name="k_nat")
            dram["v_nat"] = dscr.tile([TOK, FH], BF16, tag="vs", name="v_nat")

            attn_b0 = gsb.tile([128, HC, S], BF16, tag="attn0")
            attn_b1 = gsb.tile([128, HC, S], BF16, tag="attn1")
            attn_s = [attn_b0, attn_b1]
            mskt = gsb.tile([128, 2, 2, QT], BF16, tag="msk")
            nc.sync.dma_start(out=mskt, in_=dram["msk"])
            ones_col = gsb.tile([128, 1], BF16, tag="ones_col")
            nc.vector.memset(ones_col, 1.0)

            with ExitStack() as p1ctx:
                pools["hblk"] = p1ctx.enter_context(tc.tile_pool(name="hblk", bufs=3))
                pools["wch"] = p1ctx.enter_context(tc.tile_pool(name="wch", bufs=3))
                pools["cs"] = p1ctx.enter_context(tc.tile_pool(name="cs", bufs=5))
                pools["rtmp"] = p1ctx.enter_context(tc.tile_pool(name="rtmp", bufs=3))
                pools["stage"] = p1ctx.enter_context(tc.tile_pool(name="stage", bufs=6))

                # b0 blocks; then interleave b0 attention with b1 blocks
                for T in range(NTB // 2):
                    _emit_phase1_block(nc, T, pools, dram)
                for hh in range(HC):
                    _emit_phase2_pair(nc, 0, hh, pools, dram, attn_s, mskt,
                                      ones_col)
                    _emit_phase1_block(nc, NTB // 2 + hh, pools, dram)

            with ExitStack() as p3ctx:
                pools["wo"] = p3ctx.enter_context(tc.tile_pool(name="wo", bufs=1))
                pools["ost"] = p3ctx.enter_context(tc.tile_pool(name="ost", bufs=4))
                wot = _emit_phase3_wo(nc, pools, dram)
                # interleave b1 attention with b0-token o_proj (per qt)
                for hh in range(HC):
                    _emit_phase2_pair(
                        nc, 1, hh, pools, dram, attn_s, mskt, ones_col,
                        qt_hook=lambda qt, hh=hh: _emit_phase3_sts(
                            nc, pools, dram, attn_s, wot, [4 * hh + qt]))
                _emit_phase3_sts(nc, pools, dram, attn_s, wot, range(16, 32))

    nc.compile()
    return nc


_NC_CACHE = {}


def get_nc():
    if "nc" not in _NC_CACHE:
        _NC_CACHE["nc"] = build_nc()
    return _NC_CACHE["nc"]


def prep_in_maps(positions, hidden_states, W_qkv, W_o):
    """Host-side sharding + layout prep. Returns per-core input maps."""
    bf16 = ml_dtypes.bfloat16
    hid = np.asarray(hidden_states, np.float32).reshape(TOK, HIDDEN)
    hT = np.ascontiguousarray(hid.T).astype(bf16)

    pos = np.asarray(positions).reshape(TOK).astype(np.float32)
    half = D // 2
    inv = ROPE_BASE ** (-np.arange(half, dtype=np.float32) / half)
    ang = pos[:, None] * inv[None, :]                      # [TOK, 64]
    cos = np.cos(ang)
    sin = np.sin(ang)
    cos128 = np.concatenate([cos, cos], axis=1)            # [TOK, 128]
    sin128 = np.concatenate([-sin, sin], axis=1)
    csn = np.empty((TOK, 2, FH), np.float32)
    csn[:, 0, :] = np.tile(cos128, HC)
    csn[:, 1, :] = np.tile(sin128, HC)
    csn = csn.astype(bf16)

    kk = np.arange(128)[:, None]
    qq = np.arange(QT)[None, :]
    msk = np.stack([(qq >= kk + o * 128) for o in range(4)], axis=1)
    msk = msk.reshape(128, 2, 2, QT).astype(bf16)           # [128, 2, 2, 512]

    Wq = np.asarray(W_qkv, np.float32)
    Wo = np.asarray(W_o, np.float32)
    in_maps = []
    for c in range(N_CORES):
        wc = np.concatenate(
            [Wq[:, q0 * HIDDEN + c * FH: q0 * HIDDEN + (c + 1) * FH]
             for q0 in range(3)], axis=1).astype(bf16)
        woc = np.ascontiguousarray(Wo[c * FH:(c + 1) * FH, :]).astype(bf16)
        in_maps.append({"hT": hT, "w": wc, "wo": woc, "csn": csn, "msk": msk})
    return in_maps


def kernel(positions, hidden_states, W_qkv, W_o):
    nc = get_nc()
    in_maps = prep_in_maps(positions, hidden_states, W_qkv, W_o)
    res = run_bass_kernel_spmd(nc, in_maps, list(range(N_CORES)))
    out = res.results[0]["outp"].astype(np.float64)
    for c in range(1, N_CORES):
        out += res.results[c]["outp"]
    return out.astype(np.float32).reshape(B, S, HIDDEN)


# revision 24
# speedup vs baseline: 1.0330x; 1.0330x over previous
"""Llama attention layer (B=2, S=2048, H=4096, 32 heads, fp32 io) on 8 trn2 cores.

Sharding: tensor-parallel over heads. Each core owns 4 heads: W_qkv column
shard [4096, 3*512] (bf16), W_o row shard [512, 4096] (bf16). Each core
computes qkv proj + RoPE + causal attention for its heads + its o_proj
partial; the host sums the 8 fp32 partials (the "all-reduce").

Device kernel (per core), all matmuls bf16 with fp32 PSUM accumulation:
  phase 1: q/k/v = hiddenT-tiles.T @ W-chunks in natural [token, feat]
           layout; RoPE applied with free-dim slices; results bounced to
           DRAM scratch.
  phase 2: per (batch, head): load Q^T/K^T via DMA-transpose, V natural.
           S^T[k,q] = K^T-tile.T @ Q^T ; P = exp(SCALING*S) (no max-sub:
           |scores|<~6 so exp is safe in fp32) ; causal handled by
           multiplicative masks on 4 diagonal block offsets ; attn^T =
           V.T @ P^T accumulated over k-chunks; row sums l via ones-matmul;
           normalize by broadcast reciprocal.
  phase 3: o_partial[t, :] = attn^T-tiles.T @ W_o shard, accumulated over
           the 4 head-chunks, written as fp32.
"""

import numpy as np
import ml_dtypes

import concourse.bass as bass
import concourse.tile as tile
from concourse import bacc, mybir
from concourse.bass_utils import run_bass_kernel_spmd

# ---- problem constants (hardcoded per contract) ----
HIDDEN = 4096
NH = 32
D = 128
B = 2
S = 2048
TOK = B * S            # 4096 tokens
N_CORES = 8
HC = NH // N_CORES     # 4 heads per core
FH = HC * D            # 512 features per core for each of q/k/v
SCALING = D ** -0.5
ROPE_BASE = 10000.0

BF16 = mybir.dt.bfloat16
F32 = mybir.dt.float32

TBLK = 512             # tokens per phase-1 block
NTB = TOK // TBLK      # 8
HKC = 16               # h-dim 128-chunks per hT/w chunk tile (2 chunks = full 4096)
QT = 512               # q columns per phase-2 tile
NQT = S // QT          # 4
NKC = S // 128         # 16 k chunks per sequence


def _emit_phase1_block(nc, T, pools, dram):
    """One 512-token block: q/k/v projections (natural layout) + rope."""
    hp, wp, csp, rtp, stp, psp = (
        pools["hblk"], pools["wch"], pools["cs"], pools["rtmp"],
        pools["stage"], pools["ps"],
    )
    hT, w, csn = dram["hT"], dram["w"], dram["csn"]
    scr = [dram["q_nat"], dram["k_nat"], dram["v_nat"]]

    # hT block: [4096 h, 512 t] as 4 chunks of [128 p, 8 kc, 512 t]
    hblk = []
    for i in range(4):
        t_ = hp.tile([128, 8, TBLK], BF16, tag="hblk")
        nc.sync.dma_start(
            out=t_,
            in_=hT[i * 1024:(i + 1) * 1024, T * TBLK:(T + 1) * TBLK].rearrange(
                "(kc p) t -> p kc t", p=128),
        )
        hblk.append(t_)

    # cos/sin tiles per token subtile: [128 t, 2, 4, 128]
    csts = []
    for tt in range(4):
        cst = csp.tile([128, 2, HC, D], BF16, tag="cs")
        r0 = T * TBLK + tt * 128
        nc.sync.dma_start(
            out=cst,
            in_=csn[r0:r0 + 128, :, :].rearrange("p c (h d) -> p c h d", h=HC),
        )
        csts.append(cst)

    for j3 in range(3):  # q, k, v
        # w chunk: [4096 h, 512 f] as 4 chunks of [128 p, 8 kc, 512 f]
        wch = []
        for i in range(4):
            t_ = wp.tile([128, 8, FH], BF16, tag="wch")
            nc.sync.dma_start(
                out=t_,
                in_=w[i * 1024:(i + 1) * 1024, j3 * FH:(j3 + 1) * FH].rearrange(
                    "(kc p) f -> p kc f", p=128),
            )
            wch.append(t_)

        for tt in range(4):
            ps = psp.tile([128, HC, D], F32, tag="ps")
            for i in range(4):
                for kc in range(8):
                    nc.tensor.matmul(
                        ps,
                        lhsT=hblk[i][:, kc, tt * 128:(tt + 1) * 128],
                        rhs=wch[i][:, kc, :],
                        start=(i == 0 and kc == 0),
                        stop=(i == 3 and kc == 7),
                    )
            st = stp.tile([128, HC, D], BF16, tag="stage")
            if j3 < 2:
                cst = csts[tt]
                half = D // 2
                tr = rtp.tile([128, HC, D], F32, tag="rtmp")
                tcos = rtp.tile([128, HC, D], F32, tag="rtmp")
                # rotate-half times signed sin
                nc.vector.tensor_mul(tr[:, :, 0:half], ps[:, :, half:D], cst[:, 1, :, 0:half])
                nc.vector.tensor_mul(tr[:, :, half:D], ps[:, :, 0:half], cst[:, 1, :, half:D])
                nc.vector.tensor_mul(tcos, ps, cst[:, 0])
                nc.vector.tensor_add(st, tr, tcos)
            else:
                nc.vector.tensor_copy(out=st, in_=ps)
            r0 = T * TBLK + tt * 128
            nc.sync.dma_start(out=scr[j3][r0:r0 + 128, :], in_=st)


def _emit_phase2_pair(nc, b, hh, pools, dram, attn_s, mskt, ones_col,
                      qt_hook=None):
    """Causal attention for one (batch, local-head) pair.

    k-chunks processed in groups of 2 so one ACTIVATE covers [128, 2*QT]
    (amortizes the ~400ns per-instruction ACT overhead)."""
    pairp, ptp, pvlp, ps2p, smallp = (
        pools["pair"], pools["pt"], pools["pvl"], pools["ps2"], pools["small"],
    )
    q_nat, k_nat, v_nat = dram["q_nat"], dram["k_nat"], dram["v_nat"]
    rows = slice(b * S, (b + 1) * S)
    cols = slice(hh * D, (hh + 1) * D)

    qTp = pairp.tile([128, S], BF16, tag="pair")
    nc.sync.dma_start_transpose(out=qTp, in_=q_nat[rows, cols])
    kTp = pairp.tile([128, S], BF16, tag="pair")
    nc.sync.dma_start_transpose(out=kTp, in_=k_nat[rows, cols])
    vp = pairp.tile([128, NKC, D], BF16, tag="pair")
    nc.sync.dma_start(
        out=vp, in_=v_nat[rows, cols].rearrange("(kc p) d -> p kc d", p=128))

    for qt in range(NQT):
        qcols = slice(qt * QT, (qt + 1) * QT)
        pv = pvlp.tile([128, QT], F32, tag="pvl")
        pl = pvlp.tile([128, QT], F32, tag="pvl", name="pl")
        nkc = 4 * (qt + 1)
        for g in range(nkc // 2):
            sc = ps2p.tile([128, 2, QT], F32, tag="ps2")
            for s2 in range(2):
                kc = 2 * g + s2
                nc.tensor.matmul(
                    sc[:, s2, :], lhsT=kTp[:, kc * 128:(kc + 1) * 128],
                    rhs=qTp[:, qcols], start=True, stop=True)
            pt2 = ptp.tile([128, 2, QT], BF16, tag="pt")
            nc.scalar.activation(
                out=pt2, in_=sc, func=mybir.ActivationFunctionType.Exp,
                scale=SCALING)
            if g >= 2 * qt:  # diagonal group: apply causal mask pair
                pt2m = ptp.tile([128, 2, QT], BF16, tag="pt")
                nc.vector.tensor_mul(pt2m, pt2, mskt[:, g - 2 * qt])
                pt2 = pt2m
            for s2 in range(2):
                kc = 2 * g + s2
                nc.tensor.matmul(
                    pv, lhsT=vp[:, kc, :], rhs=pt2[:, s2, :],
                    start=(kc == 0), stop=(kc == nkc - 1))
                nc.tensor.matmul(
                    pl[0:1, :], lhsT=ones_col, rhs=pt2[:, s2, :],
                    start=(kc == 0), stop=(kc == nkc - 1))
        # normalize: attn^T[:, qcols] = pv * (1/l) broadcast over partitions
        linv = smallp.tile([1, QT], F32, tag="linv1")
        nc.vector.reciprocal_approx_fast(out=linv, in_=pl[0:1, :])
        linv_bc = smallp.tile([128, QT], F32, tag="linvbc")
        nc.gpsimd.partition_broadcast(linv_bc, linv)
        nc.vector.tensor_mul(
            attn_s[b][:, hh, qt * QT:(qt + 1) * QT], pv, linv_bc)
        if qt_hook is not None:
            qt_hook(qt)


def _emit_phase3_wo(nc, pools, dram):
    wot = pools["wo"].tile([128, HC, HIDDEN], BF16, tag="wo")
    nc.sync.dma_start(out=wot, in_=dram["wo"].rearrange("(kc p) o -> p kc o", p=128))
    return wot


def _emit_phase3_sts(nc, pools, dram, attn_s, wot, sts):
    """o_proj partial for a range of 128-token tiles."""
    ostp, psp = pools["ost"], pools["ps"]
    outp = dram["outp"]

    for st in sts:
        for ocp in range(4):  # pairs of 512-col output tiles
            pso = [psp.tile([128, QT], F32, tag="ps", name="pso0"),
                   psp.tile([128, QT], F32, tag="ps", name="pso1")]
            for kc in range(HC):
                for oc in range(2):
                    o0 = (ocp * 2 + oc) * QT
                    nc.tensor.matmul(
                        pso[oc],
                        lhsT=attn_s[st // 16][:, kc, (st % 16) * 128:
                                              (st % 16 + 1) * 128],
                        rhs=wot[:, kc, o0:o0 + QT],
                        start=(kc == 0), stop=(kc == HC - 1))
            for oc in range(2):
                o0 = (ocp * 2 + oc) * QT
                ot = ostp.tile([128, QT], F32, tag="ost")
                if oc == 0:
                    nc.vector.tensor_copy(out=ot, in_=pso[oc])
                else:
                    nc.scalar.copy(out=ot, in_=pso[oc])
                nc.sync.dma_start(
                    out=outp[st * 128:(st + 1) * 128, o0:o0 + QT], in_=ot)


def build_nc():
    nc = bacc.Bacc("TRN2", target_bir_lowering=False, debug=False,
                   num_devices=N_CORES)
    dram = {
        "hT": nc.dram_tensor("hT", [HIDDEN, TOK], BF16, kind="ExternalInput").ap(),
        "w": nc.dram_tensor("w", [HIDDEN, 3 * FH], BF16, kind="ExternalInput").ap(),
        "wo": nc.dram_tensor("wo", [FH, HIDDEN], BF16, kind="ExternalInput").ap(),
        "csn": nc.dram_tensor("csn", [TOK, 2, FH], BF16, kind="ExternalInput").ap(),
        "msk": nc.dram_tensor("msk", [128, 2, 2, QT], BF16, kind="ExternalInput").ap(),
        "outp": nc.dram_tensor("outp", [TOK, HIDDEN], F32, kind="ExternalOutput").ap(),
    }

    with tile.TileContext(nc) as tc:
        from contextlib import ExitStack
        with ExitStack() as ctx:
            # global pools (live across phases)
            pools = {}
            pools["ps"] = ctx.enter_context(tc.tile_pool(name="ps", bufs=4, space="PSUM"))
            pools["pvl"] = ctx.enter_context(tc.tile_pool(name="pvl", bufs=2, space="PSUM"))
            pools["ps2"] = ctx.enter_context(tc.tile_pool(name="ps2", bufs=1, space="PSUM"))
            gsb = ctx.enter_context(tc.tile_pool(name="gsb", bufs=1))
            pools["pair"] = ctx.enter_context(tc.tile_pool(name="pair", bufs=6))
            pools["pt"] = ctx.enter_context(tc.tile_pool(name="pt", bufs=4))
            pools["small"] = ctx.enter_context(tc.tile_pool(name="small", bufs=2))
            dscr = ctx.enter_context(tc.tile_pool(name="dscr", bufs=1, space="DRAM"))

            dram["q_nat"] = dscr.tile([TOK, FH], BF16, tag="qs", name="q_nat")
            dram["k_nat"] = dscr.tile([TOK, FH], BF16, tag="ks", name="k_nat")
            dram["v_nat"] = dscr.tile([TOK, FH], BF16, tag="vs", name="v_nat")

            attn_b0 = gsb.tile([128, HC, S], BF16, tag="attn0")
            attn_b1 = gsb.tile([128, HC, S], BF16, tag="attn1")
            attn_s = [attn_b0, attn_b1]
            mskt = gsb.tile([128, 2, 2, QT], BF16, tag="msk")
            nc.sync.dma_start(out=mskt, in_=dram["msk"])
            ones_col = gsb.tile([128, 1], BF16, tag="ones_col")
            nc.vector.memset(ones_col, 1.0)

            with ExitStack() as p1ctx:
                pools["hblk"] = p1ctx.enter_context(tc.tile_pool(name="hblk", bufs=6))
                pools["wch"] = p1ctx.enter_context(tc.tile_pool(name="wch", bufs=6))
                pools["cs"] = p1ctx.enter_context(tc.tile_pool(name="cs", bufs=5))
                pools["rtmp"] = p1ctx.enter_context(tc.tile_pool(name="rtmp", bufs=3))
                pools["stage"] = p1ctx.enter_context(tc.tile_pool(name="stage", bufs=6))

                # b0 blocks; then interleave b0 attention with b1 blocks
                for T in range(NTB // 2):
                    _emit_phase1_block(nc, T, pools, dram)
                for hh in range(HC):
                    _emit_phase2_pair(nc, 0, hh, pools, dram, attn_s, mskt,
                                      ones_col)
                    _emit_phase1_block(nc, NTB // 2 + hh, pools, dram)

            with ExitStack() as p3ctx:
                pools["wo"] = p3ctx.enter_context(tc.tile_pool(name="wo", bufs=1))
                pools["ost"] = p3ctx.enter_context(tc.tile_pool(name="ost", bufs=4))
                wot = _emit_phase3_wo(nc, pools, dram)
                # interleave b1 attention with b0-token o_proj
                for hh in range(HC):
                    _emit_phase2_pair(nc, 1, hh, pools, dram, attn_s, mskt,
                                      ones_col)
                    _emit_phase3_sts(nc, pools, dram, attn_s, wot,
                                     range(4 * hh, 4 * hh + 4))
                _emit_phase3_sts(nc, pools, dram, attn_s, wot, range(16, 32))

    nc.compile()
    return nc


_NC_CACHE = {}


def get_nc():
    if "nc" not in _NC_CACHE:
        _NC_CACHE["nc"] = build_nc()
    return _NC_CACHE["nc"]


def prep_in_maps(positions, hidden_states, W_qkv, W_o):
    """Host-side sharding + layout prep. Returns per-core input maps."""
    bf16 = ml_dtypes.bfloat16
    hid = np.asarray(hidden_states, np.float32).reshape(TOK, HIDDEN)
    hT = np.ascontiguousarray(hid.T).astype(bf16)

    pos = np.asarray(positions).reshape(TOK).astype(np.float32)
    half = D // 2
    inv = ROPE_BASE ** (-np.arange(half, dtype=np.float32) / half)
    ang = pos[:, None] * inv[None, :]                      # [TOK, 64]
    cos = np.cos(ang)
    sin = np.sin(ang)
    cos128 = np.concatenate([cos, cos], axis=1)            # [TOK, 128]
    sin128 = np.concatenate([-sin, sin], axis=1)
    csn = np.empty((TOK, 2, FH), np.float32)
    csn[:, 0, :] = np.tile(cos128, HC)
    csn[:, 1, :] = np.tile(sin128, HC)
    csn = csn.astype(bf16)

    kk = np.arange(128)[:, None]
    qq = np.arange(QT)[None, :]
    msk = np.stack([(qq >= kk + o * 128) for o in range(4)], axis=1)
    msk = msk.reshape(128, 2, 2, QT).astype(bf16)           # [128, 2, 2, 512]

    Wq = np.asarray(W_qkv, np.float32)
    Wo = np.asarray(W_o, np.float32)
    in_maps = []
    for c in range(N_CORES):
        wc = np.concatenate(
            [Wq[:, q0 * HIDDEN + c * FH: q0 * HIDDEN + (c + 1) * FH]
             for q0 in range(3)], axis=1).astype(bf16)
        woc = np.ascontiguousarray(Wo[c * FH:(c + 1) * FH, :]).astype(bf16)
        in_maps.append({"hT": hT, "w": wc, "wo": woc, "csn": csn, "msk": msk})
    return in_maps


def kernel(positions, hidden_states, W_qkv, W_o):
    nc = get_nc()
    in_maps = prep_in_maps(positions, hidden_states, W_qkv, W_o)
    res = run_bass_kernel_spmd(nc, in_maps, list(range(N_CORES)))
    out = res.results[0]["outp"].astype(np.float64)
    for c in range(1, N_CORES):
        out += res.results[c]["outp"]
    return out.astype(np.float32).reshape(B, S, HIDDEN)
